# revision 1
# baseline (speedup 1.0000x reference)
"""Multi-Head Latent Attention (B=1, S=2048, HIDDEN=2048, 16 heads, MLA_DIM=128)
as a tensor-parallel Bass kernel on 8 TRN2 NeuronCores.

Sharding: 2 heads per core (q/k/v column-shard, o_proj row-shard); the
o_proj partial sums are reduced on the host.

Device-side layout avoids all large transposes:
  - host passes hidden^T; projections produce q^T/k^T ([d, s]) and v^T
  - scores are computed transposed: s^T[sk, sq] = k^T.T-free matmul with
    lhsT=k^T, rhs=q^T (both native)
  - exp(s^T) tiles feed the PV matmul directly as lhsT, producing
    ctx[sq, d]; an appended ones-column on V yields softmax denominators
    in the same matmuls
  - ctx rows are scaled by 1/denom (per-partition), PE-transposed to
    ctx^T which is the native lhsT for the o_proj matmul.
"""

import math
import os
import sys
import time

for _p in ("/opt/trn_rl_repo", "/root/.axon_site/_ro/trn_rl_repo"):
    if os.path.isdir(_p) and _p not in sys.path:
        sys.path.append(_p)

import numpy as np

import concourse.bass as bass
import concourse.mybir as mybir
from concourse import bacc
from concourse.masks import make_identity
from concourse.tile import TileContext

B, S, HID = 1, 2048, 2048
NUM_HEADS, MLA = 16, 128
HEAD_DIM = HID // NUM_HEADS  # 128
ROPE_BASE = 10000.0
N_CORES = 8
HPC = NUM_HEADS // N_CORES  # heads per core = 2
DL = HPC * MLA              # local projection width = 256
P = 128
KT = HID // P               # 16 contraction tiles
ST = S // P                 # 16 sequence tiles
SCALE = 1.0 / math.sqrt(MLA)
VA = MLA + 1                # v columns + ones column = 129
FP = mybir.dt.float32
BF = mybir.dt.float16  # 16-bit compute dtype (fp16: better mantissa, same speed)
NEG = -1e9
EXP_BIAS = -6.0  # exp(s-6): cancels in softmax, keeps fp16 in range


def _emit(nc, tc, aps, variant):
    """Emit the per-core program. variant in ("causal", "full", "mask").

    Causal emission interleaves phases so PE never starves:
      proj(h0) | attn(h0) x proj(h1) | attn(h1) x oproj
    """
    causal = variant == "causal"
    hT, wq, wk, wv, wo, cosT, sinT, out = (
        aps["hT"], aps["wq"], aps["wk"], aps["wv"], aps["wo"],
        aps["cosT"], aps["sinT"], aps["out"])
    maskT = aps.get("maskT")

    with (
        tc.tile_pool(name="psum", bufs=1, space="PSUM") as pq,
        tc.tile_pool(name="persist", bufs=1) as pp,
        tc.tile_pool(name="work", bufs=1) as wp,
        tc.tile_pool(name="wstream", bufs=1) as ws,
    ):
        # ---- persistent SBUF ----
        qT = [pp.tile([P, S], BF, name=f"qT{h}") for h in range(HPC)]
        kT = [pp.tile([P, S], BF, name=f"kT{h}") for h in range(HPC)]
        # v (+ ones col) for both heads: s-tile t at cols [t*2*VA, ...),
        # head h at sub-cols [h*VA, h*VA+VA)
        vaug = pp.tile([P, ST * HPC * VA], BF, name="vaug")
        ctxT = [pp.tile([P, S], BF, name=f"ctxT{h}") for h in range(HPC)]
        ident = pp.tile([P, P], BF, name="ident")
        make_identity(nc, ident)
        diag = None
        if causal:
            # diag[i, j] = 0 if i <= j else -1e9   (scores^T orientation)
            diag = pp.tile([P, P], FP, name="diag")
            nc.gpsimd.memset(diag, 0.0)
            # keep where (j - i) >= 0  i.e. i <= j; else fill -1e9
            nc.gpsimd.affine_select(
                out=diag, in_=diag, compare_op=mybir.AluOpType.is_ge,
                fill=NEG, base=0, pattern=[[1, P]], channel_multiplier=-1)
        for t in range(ST):
            for h in range(HPC):
                nc.gpsimd.memset(vaug[:, t * HPC * VA + h * VA + MLA:
                                      t * HPC * VA + h * VA + VA], 1.0)
        ebias = pp.tile([P, 1], FP, name="ebias")
        nc.gpsimd.memset(ebias, EXP_BIAS)

        def load_wm(wdram, m):
            """One contiguous DMA: all 16 lhsT k-tiles for d-block m
            (host pre-tiled to [HPC*P, KT*P])."""
            wt = ws.tile([P, KT * P], BF, tag="wm", bufs=3, name="wm")
            nc.sync.dma_start(wt, wdram[m * P:(m + 1) * P, :])
            return wt

        def rope(state, sp, dst, c0, w):
            # psum fp32 -> bf16 staging on ACT, then swap-halves copies and
            # 3 full-width bf16 DVE ops.  sin_sb is sign-folded on the host
            # (rows 0:64 negated), so: dst = raw*cos + swap(raw)*sin_sg.
            cos_sb, sin_sb = state["cos_sb"], state["sin_sb"]
            cols = slice(c0, c0 + w)
            raw = wp.tile([P, w], BF, tag="qraw", bufs=3, name="raw")
            nc.scalar.copy(raw, sp)
            swp = wp.tile([P, w], BF, tag="tmpb", bufs=3, name="ropeswp")
            H2 = MLA // 2
            nc.vector.tensor_copy(swp[0:H2, :], raw[H2:P, :])
            nc.vector.tensor_copy(swp[H2:P, :], raw[0:H2, :])
            nc.vector.tensor_mul(swp, swp, sin_sb[:, cols])
            nc.vector.tensor_mul(dst[:, cols], raw, cos_sb[:, cols])
            nc.vector.tensor_add(dst[:, cols], dst[:, cols], swp)

        def mm_chunk(state, wt, c0, tag, bufs, w):
            ht_sb = state["ht_sb"]
            sp = pq.tile([P, w], FP, tag=tag, bufs=bufs, name="mmps")
            for k in range(KT):
                for c in range(w // 512):
                    nc.tensor.matmul(
                        sp[:, c * 512:(c + 1) * 512],
                        lhsT=wt[:, k * P:(k + 1) * P],
                        rhs=ht_sb[k][:, c0 + c * 512:c0 + (c + 1) * 512],
                        start=(k == 0), stop=(k == KT - 1))
            return sp

        def rope_tensor_gen(state, wsrc, h, dst, tag, bufs, w, wm_pre=None):
            wm = wm_pre if wm_pre is not None else load_wm(wsrc, h)
            for cc in range(S // w):
                sp = mm_chunk(state, wm, cc * w, tag, bufs, w)
                rope(state, sp, dst, cc * w, w)
                yield

        def v_tensor_gen(state, h, tag, bufs, w, wm_pre=None, cc0=0):
            wm = wm_pre if wm_pre is not None else load_wm(wv, h)
            for cc in range(cc0, S // w):
                sp = mm_chunk(state, wm, cc * w, tag, bufs, w)
                vt = wp.tile([P, w], BF, tag="tmpf", bufs=2, name="vtmp")
                nc.vector.tensor_copy(vt, sp)
                yield
                for b in range(w // P):
                    t = (cc * w) // P + b  # global s-tile index
                    tp = pq.tile([P, P + 1], BF, tag="small", bufs=2,
                                 name="vtp")
                    nc.tensor.transpose(tp[:, 0:P], vt[:, b * P:(b + 1) * P],
                                        ident)
                    nc.vector.tensor_copy(
                        vaug[:, t * HPC * VA + h * VA:
                             t * HPC * VA + h * VA + MLA], tp[:, 0:P])
                yield

        def chain(*gens):
            for g in gens:
                yield from g

        def head_proj_gen(state, h, tag, bufs, w, wmq_pre=None):
            return chain(
                rope_tensor_gen(state, wq, h, qT[h], tag, bufs, w,
                                wm_pre=wmq_pre),
                rope_tensor_gen(state, wk, h, kT[h], tag, bufs, w),
                v_tensor_gen(state, h, tag, bufs, w))

        # ---- attention for one head (generator) ----
        def head_attn_gen(h, after_tile=None, ep=None, lag=1):
            ep = ep if ep is not None else wp
            expT = []   # (tile, col_offset) per sk-tile
            ctxns = []

            def emit_scores(t):
                c0 = (t * P) // 1024 if causal else 0
                if causal and c0 == 1:
                    e = ep.tile([P, 1024], BF, tag="expTn", bufs=8,
                                name=f"en{t}")
                else:
                    e = ep.tile([P, S], BF, tag="expTw", bufs=8 if causal
                                else ST, name=f"ew{t}")
                eoff = c0 * 1024
                expT.append((e, eoff))
                for cc in range(c0, 2):
                    cols = slice(cc * 1024, (cc + 1) * 1024)
                    sp = pq.tile([P, 1024], FP, tag="b2", bufs=2, name="scps")
                    for c in range(2):
                        # causal: skip 512-halves entirely left of the
                        # diagonal — PV never reads cols < 128t of tile t,
                        # and exp of the (bounded) stale psum there is unread
                        if causal and cc * 1024 + (c + 1) * 512 <= t * P:
                            continue
                        nc.tensor.matmul(
                            sp[:, c * 512:(c + 1) * 512],
                            lhsT=kT[h][:, t * P:(t + 1) * P],
                            rhs=qT[h][:, cc * 1024 + c * 512:
                                      cc * 1024 + (c + 1) * 512],
                            start=True, stop=True)
                    if causal and cc == c0:
                        off = t * P - c0 * 1024
                        nc.vector.tensor_add(
                            sp[:, off:off + P], sp[:, off:off + P], diag)
                    if maskT is not None:
                        mt = ep.tile([P, 1024], FP, tag="mt", bufs=4,
                                     name="mt")
                        nc.sync.dma_start(
                            mt, maskT[t * P:(t + 1) * P, cols])
                        nc.vector.tensor_add(sp, sp, mt)
                    nc.scalar.activation(
                        e[:, cc * 1024 - eoff:(cc + 1) * 1024 - eoff], sp,
                        mybir.ActivationFunctionType.Exp, bias=ebias[:, 0:1],
                        scale=SCALE)

            def finish_tile(m):
                # PV accumulate (with ones-column denominators), normalize,
                # transpose to ctx^T
                ctx = pq.tile([P, VA], FP, tag="small", bufs=2, name="ctx")
                ks = list(range(0, m + 1)) if causal else list(range(ST))
                for k in ks:
                    ek, ekoff = expT[k]
                    nc.tensor.matmul(
                        ctx[:, 0:VA],
                        lhsT=ek[:, m * P - ekoff:(m + 1) * P - ekoff],
                        rhs=vaug[:, k * HPC * VA + h * VA:
                                 k * HPC * VA + h * VA + VA],
                        start=(k == ks[0]), stop=(k == ks[-1]))
                recip = wp.tile([P, 1], FP, tag="recip", bufs=4, name="rc")
                nc.vector.reciprocal(recip, ctx[:, MLA:VA])
                ctxn = wp.tile([P, P], BF, tag="ctxn", bufs=4, name="cn")
                nc.vector.tensor_scalar_mul(ctxn, ctx[:, 0:MLA], recip)
                tp = pq.tile([P, P + 1], BF, tag="small", bufs=2, name="ctp")
                nc.tensor.transpose(tp[:, 0:P], ctxn, ident)
                nc.vector.tensor_copy(ctxT[h][:, m * P:(m + 1) * P],
                                      tp[:, 0:P])
                if after_tile is not None:
                    after_tile(m)

            if causal:
                # PV trails scores by `lag` tiles so exp (ACT) is not waited on
                for t in range(ST):
                    emit_scores(t)
                    yield
                    if t >= lag:
                        finish_tile(t - lag)
                        yield
                for m in range(ST - lag, ST):
                    finish_tile(m)
                    yield
            else:
                for t in range(ST):
                    emit_scores(t)
                    yield
                for m in range(ST):
                    finish_tile(m)
                    yield

        # ---- output projection (row-sharded wo, partial sums) ----
        def make_oproj():
            wo_sb = []
            for h in range(HPC):
                wt = wp.tile([P, HID], BF, tag="wosb", bufs=2, name=f"wo{h}")
                nc.sync.dma_start(wt, wo[h * P:(h + 1) * P, :])
                wo_sb.append(wt)

            def oproj_m(m):
                for q4 in range(4):
                    op = pq.tile([P, 512], FP, tag="b2p", bufs=2, name="ops")
                    for h in range(HPC):
                        nc.tensor.matmul(
                            op, lhsT=ctxT[h][:, m * P:(m + 1) * P],
                            rhs=wo_sb[h][:, q4 * 512:(q4 + 1) * 512],
                            start=(h == 0), stop=(h == HPC - 1))
                    ob = wp.tile([P, 512], BF, tag="ob", bufs=6, name="ob")
                    act_share = 2 if m >= ST - 6 else 1  # ACT free after exp
                    if q4 < act_share:
                        nc.scalar.copy(ob, op)
                    else:
                        nc.vector.tensor_copy(ob, op)
                    nc.sync.dma_start(
                        out[m * P:(m + 1) * P, q4 * 512:(q4 + 1) * 512], ob)
            return oproj_m

        def load_resident(hp, mid_cb=None):
            state = {}
            ht_sb = []
            for k in range(KT):
                ht = hp.tile([P, S], BF, name=f"ht{k}")
                eng = nc.scalar if k % 2 == 0 else nc.sync
                eng.dma_start(ht, hT[k * P:(k + 1) * P, :])
                ht_sb.append(ht)
                if k == 0 and mid_cb is not None:
                    mid_cb()
            state["ht_sb"] = ht_sb
            cos_sb = hp.tile([P, S], BF, name="cos_sb")
            sin_sb = hp.tile([P, S], BF, name="sin_sb")
            nc.scalar.dma_start(cos_sb, cosT[:, :])
            nc.scalar.dma_start(sin_sb, sinT[:, :])
            state["cos_sb"] = cos_sb
            state["sin_sb"] = sin_sb
            return state

        def run(gen):
            for _ in gen:
                pass

        def interleave(ga, gb, ratio=2):
            """Drive `ratio` units of ga per unit of gb until both end."""
            alive_a, alive_b = True, True
            while alive_a or alive_b:
                for _ in range(ratio):
                    if alive_a:
                        try:
                            next(ga)
                        except StopIteration:
                            alive_a = False
                if alive_b:
                    try:
                        next(gb)
                    except StopIteration:
                        alive_b = False

        if causal:
            with tc.tile_pool(name="htp", bufs=1) as hp:
                # q0 weights ahead of hT; k0 weights right after ht[0]
                wmq0 = load_wm(wq, 0)
                wmk0 = [None]
                state = load_resident(
                    hp, mid_cb=lambda: wmk0.__setitem__(0, load_wm(wk, 0)))
                wmk0 = wmk0[0]
                qs = [pq.tile([P, 1024], FP, tag="b2", bufs=2,
                              name=f"q0s{i}") for i in range(2)]
                kss = [pq.tile([P, 512], FP, tag="b2p", bufs=2,
                               name=f"k0s{i}") for i in range(2)]
                ht_sb = state["ht_sb"]
                for k in range(KT):
                    st, sp_ = (k == 0), (k == KT - 1)
                    for i in range(2):
                        nc.tensor.matmul(
                            kss[i], lhsT=wmk0[:, k * P:(k + 1) * P],
                            rhs=ht_sb[k][:, i * 512:(i + 1) * 512],
                            start=st, stop=sp_)
                    for i in range(2):
                        for c in range(2):
                            nc.tensor.matmul(
                                qs[i][:, c * 512:(c + 1) * 512],
                                lhsT=wmq0[:, k * P:(k + 1) * P],
                                rhs=ht_sb[k][:, i * 1024 + c * 512:
                                             i * 1024 + (c + 1) * 512],
                                start=st, stop=sp_)
                for i in range(2):
                    rope(state, kss[i], kT[0], i * 512, 512)
                for i in range(2):
                    rope(state, qs[i], qT[0], i * 1024, 1024)
                for cc in range(2, 4):
                    sp = mm_chunk(state, wmk0, cc * 512, "b2p", 2, 512)
                    rope(state, sp, kT[0], cc * 512, 512)
                run(v_tensor_gen(state, 0, "b2", 2, 1024))
                interleave(head_attn_gen(0),
                           head_proj_gen(state, 1, "b2p", 2, 512))
                oproj_m = make_oproj()
                run(head_attn_gen(1, after_tile=oproj_m, lag=2))
        else:
            with tc.tile_pool(name="htp", bufs=1) as hp:
                wmq0 = load_wm(wq, 0)
                state = load_resident(hp)
                run(head_proj_gen(state, 0, "b2", 2, 1024, wmq_pre=wmq0))
                run(head_proj_gen(state, 1, "b2", 2, 1024))
            with tc.tile_pool(name="expp", bufs=1) as ep:
                run(head_attn_gen(0, ep=ep))
                oproj_m = make_oproj()
                run(head_attn_gen(1, after_tile=oproj_m, ep=ep))


def _build(variant):
    nc = bacc.Bacc("TRN2", target_bir_lowering=False, debug=False,
                   enable_asserts=False, num_devices=N_CORES)
    aps = {
        "hT": nc.dram_tensor("hT", [HID, S], BF, kind="ExternalInput").ap(),
        "wq": nc.dram_tensor("wq", [DL, KT * P], BF, kind="ExternalInput").ap(),
        "wk": nc.dram_tensor("wk", [DL, KT * P], BF, kind="ExternalInput").ap(),
        "wv": nc.dram_tensor("wv", [DL, KT * P], BF, kind="ExternalInput").ap(),
        "wo": nc.dram_tensor("wo", [DL, HID], BF, kind="ExternalInput").ap(),
        "cosT": nc.dram_tensor("cosT", [MLA, S], BF, kind="ExternalInput").ap(),
        "sinT": nc.dram_tensor("sinT", [MLA, S], BF, kind="ExternalInput").ap(),
        "out": nc.dram_tensor("out", [S, HID], BF, kind="ExternalOutput").ap(),
    }
    if variant == "mask":
        aps["maskT"] = nc.dram_tensor("maskT", [S, S], FP,
                                      kind="ExternalInput").ap()
    with TileContext(nc) as tc:
        _emit(nc, tc, aps, variant)
    nc.compile()
    return nc


def _rope_tables():
    inv = (1.0 / (ROPE_BASE ** (np.arange(0, MLA, 2, dtype=np.float32) / MLA)))
    t = np.arange(S, dtype=np.float32)
    freqs = np.outer(t, inv).astype(np.float32)          # [S, 64]
    emb = np.concatenate([freqs, freqs], axis=-1)        # [S, 128]
    cosT = np.ascontiguousarray(np.cos(emb).astype(np.float32).T)
    sinT = np.ascontiguousarray(np.sin(emb).astype(np.float32).T)
    sinT[0:MLA // 2, :] *= -1.0  # sign-fold for the swap-halves rope form
    return cosT, sinT


_CAUSAL_REF = None


def _detect_variant(mask2d):
    global _CAUSAL_REF
    if not mask2d.any():
        return "full"
    if _CAUSAL_REF is None:
        _CAUSAL_REF = np.where(
            np.tril(np.ones((S, S), dtype=bool)), np.float32(0.0),
            np.float32(NEG)).astype(np.float32)
    if np.array_equal(mask2d, _CAUSAL_REF):
        return "causal"
    return "mask"


def _make_in_maps(hidden, wq, wk, wv, wo, mask2d, variant):
    bf = np.float16
    hTn = np.ascontiguousarray(hidden.reshape(S, HID).T).astype(bf)
    cosT, sinT = _rope_tables()
    cosT, sinT = cosT.astype(bf), sinT.astype(bf)
    wqb, wkb, wvb = wq.astype(bf), wk.astype(bf), wv.astype(bf)
    wob = wo.astype(bf)

    def pretile(w, c):
        # [HID, DL] slice -> [HPC*P, KT*P]: row m*P+p, col k*P+cc holds
        # w[k*P+p, c*DL + m*P + cc]  (lhsT k-tiles laid out contiguously)
        ws_ = w[:, c * DL:(c + 1) * DL]
        return np.ascontiguousarray(
            ws_.reshape(KT, P, HPC, P).transpose(2, 1, 0, 3).reshape(
                HPC * P, KT * P))

    maps = []
    for c in range(N_CORES):
        m = {
            "hT": hTn,
            "wq": pretile(wqb, c),
            "wk": pretile(wkb, c),
            "wv": pretile(wvb, c),
            "wo": np.ascontiguousarray(wob[c * DL:(c + 1) * DL, :]),
            "cosT": cosT,
            "sinT": sinT,
        }
        if variant == "mask":
            m["maskT"] = np.ascontiguousarray(mask2d.T) * np.float32(1.0 / SCALE)
        maps.append(m)
    return maps


class Runner:
    """Compiled program + reusable jitted sharded executable."""

    def __init__(self, variant):
        self.variant = variant
        self.nc = _build(variant)
        self._jit = None
        self._meta = None

    def _prep(self):
        import jax
        from jax.sharding import Mesh, NamedSharding, PartitionSpec
        from jax.experimental.shard_map import shard_map
        from concourse import bass2jax
        from concourse.bass2jax import _bass_exec_p, install_neuronx_cc_hook

        from concourse.bass2jax import partition_id_tensor

        install_neuronx_cc_hook()
        nc = self.nc
        part_name = (nc.partition_id_tensor.name
                     if nc.partition_id_tensor else None)
        in_names, out_names, out_avals = [], [], []
        for alloc in nc.m.functions[0].allocations:
            if not isinstance(alloc, mybir.MemoryLocationSet):
                continue
            name = alloc.memorylocations[0].name
            if alloc.kind == "ExternalInput":
                if name != part_name:
                    in_names.append(name)
            elif alloc.kind == "ExternalOutput":
                out_names.append(name)
                out_avals.append(jax.core.ShapedArray(
                    tuple(alloc.tensor_shape), mybir.dt.np(alloc.dtype)))
        n_params = len(in_names)
        all_names = in_names + out_names
        if part_name is not None:
            all_names = all_names + [part_name]

        def _body(*args):
            operands = list(args)
            if part_name is not None:
                operands.append(partition_id_tensor())
            outs = _bass_exec_p.bind(
                *operands, out_avals=tuple(out_avals),
                in_names=tuple(all_names),
                out_names=tuple(out_names), lowering_input_output_aliases=(),
                sim_require_finite=True, sim_require_nnan=True, nc=nc)
            return tuple(outs)

        devices = jax.devices()[:N_CORES]
        mesh = Mesh(np.asarray(devices), ("core",))
        nsh = NamedSharding(mesh, PartitionSpec("core"))
        n_outs = len(out_names)
        jitted = jax.jit(
            shard_map(_body, mesh=mesh,
                      in_specs=(PartitionSpec("core"),) * (n_params + n_outs),
                      out_specs=(PartitionSpec("core"),) * n_outs,
                      check_rep=False),
            donate_argnums=tuple(range(n_params, n_params + n_outs)),
            keep_unused=True)
        self._jit = jitted
        self._meta = (in_names, out_names, out_avals, nsh)

    def run(self, in_maps):
        """One execution; returns list of per-core output dicts."""
        import jax
        if self._jit is None:
            self._prep()
        in_names, out_names, out_avals, nsh = self._meta
        concat_in = [
            jax.device_put(
                np.concatenate([m[n] for m in in_maps], axis=0), nsh)
            for n in in_names]
        zeros = [
            jax.device_put(
                np.zeros((N_CORES * a.shape[0], *a.shape[1:]), a.dtype), nsh)
            for a in out_avals]
        outs = self._jit(*concat_in, *zeros)
        outs = [np.asarray(o) for o in outs]
        return [
            {n: outs[i].reshape(N_CORES, *out_avals[i].shape)[c]
             for i, n in enumerate(out_names)}
            for c in range(N_CORES)]

    def time_exec(self, in_maps, iters=20):
        """Median wall-clock seconds per on-device execution (inputs staged
        on device once; fresh donated output buffers pre-staged per iter)."""
        import jax
        if self._jit is None:
            self._prep()
        in_names, out_names, out_avals, nsh = self._meta
        concat_in = [
            jax.device_put(
                np.concatenate([m[n] for m in in_maps], axis=0), nsh)
            for n in in_names]
        zero_np = [np.zeros((N_CORES * a.shape[0], *a.shape[1:]), a.dtype)
                   for a in out_avals]
        # warmup (compiles)
        out = self._jit(*concat_in, *[jax.device_put(z, nsh) for z in zero_np])
        jax.block_until_ready(out)
        times = []
        for _ in range(iters):
            zs = [jax.device_put(z, nsh) for z in zero_np]
            jax.block_until_ready(zs)
            t0 = time.perf_counter()
            out = self._jit(*concat_in, *zs)
            jax.block_until_ready(out)
            times.append(time.perf_counter() - t0)
        return float(np.median(times))


_RUNNERS = {}


def _get_runner(variant):
    if variant not in _RUNNERS:
        _RUNNERS[variant] = Runner(variant)
    return _RUNNERS[variant]


def kernel(hidden_states, wq, wk, wv, wo, attention_mask):
    hidden_states = np.asarray(hidden_states, dtype=np.float32)
    wq = np.asarray(wq, dtype=np.float32)
    wk = np.asarray(wk, dtype=np.float32)
    wv = np.asarray(wv, dtype=np.float32)
    wo = np.asarray(wo, dtype=np.float32)
    mask2d = np.asarray(attention_mask, dtype=np.float32)[0, 0]
    assert hidden_states.shape == (B, S, HID)

    variant = _detect_variant(mask2d)
    runner = _get_runner(variant)
    in_maps = _make_in_maps(hidden_states, wq, wk, wv, wo, mask2d, variant)
    results = runner.run(in_maps)
    acc = np.zeros((S, HID), dtype=np.float64)
    for c in range(N_CORES):
        acc += results[c]["out"]
    return acc.astype(np.float32).reshape(B, S, HID)

